# revision 6
# baseline (speedup 1.0000x reference)
"""GAT node-level layer on 8 TRN2 NeuronCores — v2 (dma_gather edition).

Strategy (dst-sharded, edge processing per 128-dst window):
 - Host (index-only): sort edges by dst, shard by dst range (6250/core),
   window = 128 consecutive dst. Within each window split edges by src
   half (<25000 / >=25000) so gather indices fit int16. Windows are
   grouped into super-windows of SW; per super-window ONE dma_gather per
   half fetches all edge rows (512B each) from the replicated node table.
 - Node table row (512B = 256 bf16): [z(128) | s_hi | s_lo | 1.0 | pad].
   Built locally per shard (131 cols) and AllGathered with a strided
   output AP directly into the padded layout.
 - Per-edge e = leaky_relu(s_src + q_dst):
     s_src rides the gathered row (hi+lo bf16 split);
     q_dst via ONE fused scalar_tensor_tensor per chunk:
       accum_out[p] = sum_d (iota[d]==seg[p])*q_repl[p,d] = q[seg[p]],
     where q_repl (each row = the window's 128 q values) is materialized
     by a 0-stride broadcast DMA read of a [wpc,128] DRAM bounce.
 - Selection: per chunk ONE fused tensor_scalar (iota==seg)*exp(e) -> S;
   ONE PE matmul S.T @ row[0:131] accumulates z-sum AND denominator
   (table's 1.0 column) in PSUM. Normalize, DMA out.
"""

import sys

if "/opt/trn_rl_repo" not in sys.path:
    sys.path.insert(0, "/opt/trn_rl_repo")

from contextlib import ExitStack

import numpy as np

from concourse import bacc, bass, library_config, mybir, tile
from concourse.masks import make_identity

N_NODES = 50000
N_EDGES = 800000
D_IN = 256
D_OUT = 128
CORES = 8
P = 128
SW = 3  # windows per super-window

F32 = mybir.dt.float32
BF16 = mybir.dt.bfloat16
I32 = mybir.dt.int32
I16 = mybir.dt.int16
NP_BF16 = mybir.dt.np(BF16)

ROW = 2 * D_OUT  # bf16 elems per padded table row (512B)
NCOL = D_OUT + 3  # used cols: z(128) | s_hi | s_lo | one

_PROGRAM_CACHE: dict = {}


# ---------------------------------------------------------------- host prep
def preprocess_indices(src, dst, n_nodes=N_NODES, cores=CORES, sw=SW):
    """Integer-only host preprocessing. Returns static layout metadata and
    per-core index/segment arrays."""
    shard = n_nodes // cores
    wpc = (shard + P - 1) // P
    half = n_nodes // 2
    src = np.asarray(src).astype(np.int64)
    dst = np.asarray(dst).astype(np.int64)

    order = np.argsort(dst, kind="stable")
    ds = dst[order]
    ss = src[order]
    bounds = np.searchsorted(ds, np.arange(cores + 1) * shard)

    # per (core, window): (srcA, relA, srcB, relB)
    per = {}
    for c in range(cores):
        lo, hi = int(bounds[c]), int(bounds[c + 1])
        dloc = ds[lo:hi] - c * shard
        s_c = ss[lo:hi]
        w = (dloc >> 7).astype(np.int64)
        wstarts = np.searchsorted(w, np.arange(wpc + 1))
        for wi in range(wpc):
            a, b = int(wstarts[wi]), int(wstarts[wi + 1])
            sw_src = s_c[a:b]
            sw_rel = dloc[a:b] & 127
            selA = sw_src < half
            per[(c, wi)] = (
                sw_src[selA],
                sw_rel[selA],
                sw_src[~selA] - half,
                sw_rel[~selA],
            )

    capA = np.zeros(wpc, np.int64)
    capB = np.zeros(wpc, np.int64)
    for wi in range(wpc):
        for c in range(cores):
            sa, _, sb, _ = per[(c, wi)]
            capA[wi] = max(capA[wi], (len(sa) + P - 1) // P)
            capB[wi] = max(capB[wi], (len(sb) + P - 1) // P)

    supers = []  # list of dicts with static layout
    w0 = 0
    while w0 < wpc:
        ws = list(range(w0, min(w0 + sw, wpc)))
        cA = [int(capA[w]) for w in ws]
        cB = [int(capB[w]) for w in ws]
        nA = sum(cA)
        nB = sum(cB)
        cap_s = nA + nB
        offA = np.concatenate([[0], np.cumsum(cA)])[:-1]
        offB = nA + np.concatenate([[0], np.cumsum(cB)])[:-1]
        supers.append(
            dict(
                ws=ws,
                capA=cA,
                capB=cB,
                nA=nA,
                nB=nB,
                cap=cap_s,
                offA=[int(x) for x in offA],
                offB=[int(x) for x in offB],
            )
        )
        w0 += sw

    tot_cols = sum(s["cap"] for s in supers)
    tot_idx16 = sum((s["nA"] + s["nB"]) * P // 16 for s in supers)

    # per-core arrays
    arrs = []
    for c in range(cores):
        idx_arr = np.zeros((P, tot_idx16), np.int16)
        seg_arr = np.full((P, tot_cols), -1.0, np.float32)
        colbase = 0
        idxbase = 0  # in int16 columns (16 idx per column)
        for s in supers:
            cap_s = s["cap"]
            for part, (nch_key, off_key, sel) in enumerate(
                [("capA", "offA", 0), ("capB", "offB", 2)]
            ):
                stream = np.zeros(sum(s[nch_key]) * P, np.int64)
                for k, w in enumerate(s["ws"]):
                    srcs = per[(c, w)][sel]
                    rels = per[(c, w)][sel + 1]
                    coff = s[off_key][k]  # col offset within super
                    n = len(srcs)
                    base = (coff - (0 if part == 0 else s["nA"])) * P
                    stream[base : base + n] = srcs
                    j = np.arange(n)
                    seg_arr[j & 127, colbase + coff + (j >> 7)] = rels
                npos = len(stream)
                if npos:
                    wrapped = stream.reshape(npos // 16, 16).T  # [16, npos/16]
                    # HW: each of the 8 Q7 cores reads its own 16-partition
                    # group; indices must be replicated into all groups.
                    for kq in range(8):
                        idx_arr[16 * kq : 16 * (kq + 1), idxbase : idxbase + npos // 16] = wrapped
                    idxbase += npos // 16
            colbase += cap_s
        arrs.append({"idx_d": idx_arr, "seg_d": seg_arr})

    meta = dict(
        supers=supers,
        tot_cols=tot_cols,
        tot_idx16=tot_idx16,
        wpc=wpc,
        shard=shard,
        half=half,
    )
    return meta, arrs


# ---------------------------------------------------------------- program
def build_program(meta, n_nodes=N_NODES, d_in=D_IN, d_out=D_OUT, cores=CORES):
    shard = meta["shard"]
    wpc = meta["wpc"]
    half = meta["half"]
    supers = meta["supers"]
    tot_cols = meta["tot_cols"]
    tot_idx16 = meta["tot_idx16"]
    kc_n = d_in // P

    nc = bacc.Bacc(None, target_bir_lowering=False, debug=False)

    h_t = nc.dram_tensor("h_t", [d_in, shard], F32, kind="ExternalInput")
    w_d = nc.dram_tensor("W", [d_out, d_in], F32, kind="ExternalInput")
    a_d = nc.dram_tensor("a", [2 * d_out, 1], F32, kind="ExternalInput")
    idx_d = nc.dram_tensor("idx_d", [P, tot_idx16], I16, kind="ExternalInput")
    seg_d = nc.dram_tensor("seg_d", [P, tot_cols], F32, kind="ExternalInput")
    out_d = nc.dram_tensor("out", [shard, d_out], F32, kind="ExternalOutput")

    rg = [list(range(cores))]

    with tile.TileContext(nc) as tc:
        with ExitStack() as ctx:
            dram = ctx.enter_context(tc.tile_pool(name="dram", bufs=1, space="DRAM"))
            tab_loc = dram.tile([shard, ROW], BF16)
            tab_full = dram.tile([n_nodes, ROW], BF16, addr_space="Shared")
            q_bounce = dram.tile([wpc, P], BF16)

            const = ctx.enter_context(tc.tile_pool(name="const", bufs=1))

            # ---- constants
            identity = const.tile([P, P], F32)
            make_identity(nc, identity[:])
            identity_bf = const.tile([P, P], BF16)
            nc.vector.tensor_copy(identity_bf[:], identity[:])
            iota_i = const.tile([P, P], I32)
            nc.gpsimd.iota(iota_i[:], pattern=[[1, P]], base=0, channel_multiplier=0)
            iota_f = const.tile([P, P], BF16)
            nc.vector.tensor_copy(iota_f[:], iota_i[:])

            q_sb = const.tile([P, wpc], BF16)
            nc.vector.memset(q_sb[:], 0.0)

            w_sb = const.tile([P, d_in], F32)
            nc.sync.dma_start(out=w_sb[:], in_=w_d[:, :])
            a_sb = const.tile([P, 2], F32)
            nc.sync.dma_start(out=a_sb[:, 0:1], in_=a_d[0:P, :])
            nc.sync.dma_start(out=a_sb[:, 1:2], in_=a_d[P : 2 * P, :])

            # ---- phase 1: z_aug = h_shard @ [W.T | a_src | a_dst]
            ctx1 = ctx.enter_context(ExitStack())
            ph1 = ctx1.enter_context(tc.tile_pool(name="ph1", bufs=1))

            waug = ph1.tile([P, kc_n, d_out + 2], BF16)
            with tc.tile_pool(name="psum_w", bufs=2, space="PSUM") as psum_w:
                for kc in range(kc_n):
                    ksl = slice(kc * P, (kc + 1) * P)
                    pt = psum_w.tile([P, P], F32, tag="pt")
                    nc.tensor.transpose(pt[:], w_sb[:, ksl], identity[:])
                    nc.vector.tensor_copy(waug[:, kc, 0:d_out], pt[:])
                    pv = psum_w.tile([P, 2], F32, tag="pv")
                    nc.tensor.matmul(
                        out=pv[:, 0:1], lhsT=w_sb[:, ksl], rhs=a_sb[:, 0:1],
                        start=True, stop=True,
                    )
                    nc.tensor.matmul(
                        out=pv[:, 1:2], lhsT=w_sb[:, ksl], rhs=a_sb[:, 1:2],
                        start=True, stop=True,
                    )
                    nc.vector.tensor_copy(waug[:, kc, d_out : d_out + 2], pv[:])
            psum1 = ctx1.enter_context(tc.tile_pool(name="psum1", bufs=3, space="PSUM"))

            h_sb = ph1.tile([P, kc_n, shard], BF16)
            hq = (shard + 3) // 4
            for kc in range(kc_n):
                for qi in range(4):
                    c0 = qi * hq
                    c1 = min(shard, c0 + hq)
                    nc.gpsimd.dma_start(
                        out=h_sb[:, kc, c0:c1],
                        in_=h_t[kc * P : (kc + 1) * P, c0:c1],
                    )

            t_all = ph1.tile([P, 6, ROW], BF16)
            # pad cols + ones col are invariant across rotation: init once.
            nc.vector.memset(t_all[:], 0.0)
            nc.vector.memset(t_all[:, :, d_out + 2 : d_out + 3], 1.0)
            for nt in range(wpc):
                n0 = nt * P
                rows = min(P, shard - n0)
                pz = psum1.tile([P, d_out + 2], F32, tag="pz")
                for kc in range(kc_n):
                    nc.tensor.matmul(
                        out=pz[0:rows, :],
                        lhsT=h_sb[:, kc, n0 : n0 + rows],
                        rhs=waug[:, kc, :],
                        start=(kc == 0),
                        stop=(kc == kc_n - 1),
                    )
                t = t_all[:, nt % 6, :]
                # z and s_hi in one copy (pz cols 0:129 -> t cols 0:129)
                nc.vector.tensor_copy(
                    t[0:rows, 0 : d_out + 1], pz[0:rows, 0 : d_out + 1]
                )
                nc.vector.tensor_tensor(
                    out=t[0:rows, d_out + 1 : d_out + 2],
                    in0=pz[0:rows, d_out : d_out + 1],
                    in1=t[0:rows, d_out : d_out + 1],
                    op=mybir.AluOpType.subtract,
                )
                # q (bf16)
                nc.vector.tensor_copy(
                    q_sb[0:rows, nt : nt + 1], pz[0:rows, d_out + 1 : d_out + 2]
                )
                # batch the table writeback: 3 full windows per DMA (the
                # 49 x 500ns HWDGE fixed costs otherwise gate the collective)
                if rows == P and nt % 3 == 2:
                    g0 = nt - 2
                    s0 = g0 % 6
                    nc.sync.dma_start(
                        out=tab_loc[g0 * P : g0 * P + 3 * P, :].rearrange(
                            "(g p) c -> p g c", g=3
                        ),
                        in_=t_all[:, s0 : s0 + 3, :],
                    )
                elif rows < P or nt == wpc - 1:
                    # flush any unflushed full windows of this group, then
                    # this window, individually
                    for bt in range(nt - nt % 3, nt + 1):
                        b0 = bt * P
                        brows = min(P, shard - b0)
                        nc.sync.dma_start(
                            out=tab_loc[b0 : b0 + brows, :],
                            in_=t_all[0:brows, bt % 6, :],
                        )

            # q transposed to DRAM: q_bounce[w, :] = q_sb[:, w]
            qtp = psum1.tile([P, P], BF16, tag="qtp")
            nc.tensor.transpose(qtp[0:wpc, :], q_sb[:, 0:wpc], identity_bf[:])
            qt_sb = ph1.tile([P, P], BF16)
            nc.vector.tensor_copy(qt_sb[0:wpc, :], qtp[0:wpc, :])
            nc.sync.dma_start(out=q_bounce[:, :], in_=qt_sb[0:wpc, :])

            ctx1.close()

            # ---- phase 1.5: everything that doesn't need the table.
            # q_repl: one broadcast-read DMA; qv lookups for ALL chunks run
            # on DVE while the AllGather is in flight.
            nc.gpsimd.load_library(library_config.mlp)
            ip = ctx.enter_context(tc.tile_pool(name="ip", bufs=1))
            idx_sb = ip.tile([P, tot_idx16], I16)
            nc.sync.dma_start(out=idx_sb[:], in_=idx_d[:, :])
            seg_sb = ip.tile([P, tot_cols], F32)
            nc.sync.dma_start(out=seg_sb[:], in_=seg_d[:, :])
            qr_all = ip.tile([P, wpc, P], BF16)
            nc.sync.dma_start(
                out=qr_all[:],
                in_=q_bounce[:, :].unsqueeze(0).to_broadcast([P, wpc, P]),
            )
            qv_all = ip.tile([P, tot_cols], F32)

            gp = ctx.enter_context(tc.tile_pool(name="gp", bufs=3))
            ep = ctx.enter_context(tc.tile_pool(name="ep", bufs=2))
            jp = ctx.enter_context(tc.tile_pool(name="jp", bufs=4))
            sp = ctx.enter_context(tc.tile_pool(name="sp", bufs=8))
            op = ctx.enter_context(tc.tile_pool(name="op", bufs=2))
            psum_o = ctx.enter_context(
                tc.tile_pool(name="psum_o", bufs=3, space="PSUM")
            )

            n_sup = len(supers)
            colbase = [0] * n_sup
            idxbase = [0] * n_sup
            cb = ib = 0
            for si, s in enumerate(supers):
                colbase[si] = cb
                idxbase[si] = ib
                cb += s["cap"]
                ib += (s["nA"] + s["nB"]) * P // 16

            # qv[p, col] = q_window[seg[p, col]] for every chunk, hidden
            # under the AllGather.
            for si, s in enumerate(supers):
                cbase = colbase[si]
                for k, w in enumerate(s["ws"]):
                    cols = [s["offA"][k] + i for i in range(s["capA"][k])] + [
                        s["offB"][k] + i for i in range(s["capB"][k])
                    ]
                    for col in cols:
                        junk = jp.tile([P, P], BF16, tag="junk")
                        nc.vector.scalar_tensor_tensor(
                            out=junk[:],
                            in0=iota_f[:],
                            scalar=seg_sb[:, cbase + col : cbase + col + 1],
                            in1=qr_all[:, w, :],
                            op0=mybir.AluOpType.is_equal,
                            op1=mybir.AluOpType.mult,
                            accum_out=qv_all[:, cbase + col : cbase + col + 1],
                        )

            nc.gpsimd.collective_compute(
                "AllGather",
                mybir.AluOpType.bypass,
                replica_groups=rg,
                ins=[tab_loc[:, :]],
                outs=[tab_full[:, :]],
            )

            g_t = [None] * n_sup

            def emit_stage_a(si):
                """gathers for super si."""
                s = supers[si]
                cap_s = s["cap"]

                g = gp.tile([P, cap_s, ROW], BF16, tag="g")
                ibase = idxbase[si]
                nA, nB = s["nA"], s["nB"]
                if nA:
                    nc.gpsimd.dma_gather(
                        out_ap=g[:, 0:nA, :],
                        in_ap=tab_full[0:half, :],
                        idxs_ap=idx_sb[:, ibase : ibase + nA * 8],
                        num_idxs=nA * P,
                        num_idxs_reg=nA * P,
                        elem_size=ROW,
                        single_packet=False,
                    )
                if nB:
                    nc.gpsimd.dma_gather(
                        out_ap=g[:, nA : nA + nB, :],
                        in_ap=tab_full[half : 2 * half, :],
                        idxs_ap=idx_sb[:, ibase + nA * 8 : ibase + (nA + nB) * 8],
                        num_idxs=nB * P,
                        num_idxs_reg=nB * P,
                        elem_size=ROW,
                        single_packet=False,
                    )
                g_t[si] = g

            def emit_stage_b(si):
                """e, exp, selection, main matmuls, normalize, out."""
                s = supers[si]
                cap_s = s["cap"]
                cbase = colbase[si]
                g = g_t[si]

                # per-window col lists
                wcols = []
                for k in range(len(s["ws"])):
                    wcols.append(
                        [s["offA"][k] + i for i in range(s["capA"][k])]
                        + [s["offB"][k] + i for i in range(s["capB"][k])]
                    )

                sv = ep.tile([P, cap_s, 1], F32, tag="sv")
                nc.vector.tensor_tensor(
                    out=sv[:],
                    in0=g[:, :, d_out : d_out + 1],
                    in1=g[:, :, d_out + 1 : d_out + 2],
                    op=mybir.AluOpType.add,
                )
                x = ep.tile([P, cap_s], F32, tag="x")
                nc.vector.tensor_tensor(
                    out=x[:],
                    in0=sv[:, :, 0],
                    in1=qv_all[:, cbase : cbase + cap_s],
                    op=mybir.AluOpType.add,
                )
                xm = ep.tile([P, cap_s], F32, tag="xm")
                nc.vector.scalar_tensor_tensor(
                    out=xm[:],
                    in0=x[:],
                    scalar=0.01,
                    in1=x[:],
                    op0=mybir.AluOpType.mult,
                    op1=mybir.AluOpType.max,
                )
                ex = ep.tile([P, cap_s], F32, tag="ex")
                nc.scalar.activation(
                    out=ex[:], in_=xm[:], func=mybir.ActivationFunctionType.Exp
                )

                # each [NCOL] f32 slice must stay inside one 2KB PSUM bank:
                # put at most 3 windows per PSUM tile.
                po_a = psum_o.tile([P, 3, NCOL], F32, tag="po_a")
                def po_slice(k):
                    return po_a[:, k, :]
                for k, w in enumerate(s["ws"]):
                    cols = wcols[k]
                    for j, col in enumerate(cols):
                        ssel = sp.tile([P, P], BF16, tag="ssel")
                        nc.vector.tensor_scalar(
                            out=ssel[:],
                            in0=iota_f[:],
                            scalar1=seg_sb[:, cbase + col : cbase + col + 1],
                            scalar2=ex[:, col : col + 1],
                            op0=mybir.AluOpType.is_equal,
                            op1=mybir.AluOpType.mult,
                        )
                        nc.tensor.matmul(
                            out=po_slice(k),
                            lhsT=ssel[:],
                            rhs=g[:, col, 0:NCOL],
                            start=(j == 0),
                            stop=(j == len(cols) - 1),
                        )

                nw = len(s["ws"])
                ot_all = op.tile([P, nw, d_out], F32, tag="ot_all")
                all_full = all(
                    min(P, shard - w * P) == P for w in s["ws"]
                )
                for k, w in enumerate(s["ws"]):
                    den = ep.tile([P, 1], F32, tag="den")
                    nc.scalar.activation(
                        out=den[:],
                        in_=po_slice(k)[:, d_out + 2 : d_out + 3],
                        func=mybir.ActivationFunctionType.Copy,
                        bias=1e-6,
                    )
                    rec = ep.tile([P, 1], F32, tag="rec")
                    nc.vector.reciprocal(rec[:], den[:])
                    nc.scalar.activation(
                        out=ot_all[:, k, :],
                        in_=po_slice(k)[:, 0:d_out],
                        func=mybir.ActivationFunctionType.Copy,
                        scale=rec[:, 0:1],
                    )
                if all_full:
                    w0 = s["ws"][0] * P
                    nc.sync.dma_start(
                        out=out_d[w0 : w0 + nw * P, :].rearrange(
                            "(k p) c -> p k c", k=nw
                        ),
                        in_=ot_all[:, 0:nw, :],
                    )
                else:
                    for k, w in enumerate(s["ws"]):
                        n0 = w * P
                        rows = min(P, shard - n0)
                        nc.sync.dma_start(
                            out=out_d[n0 : n0 + rows, :],
                            in_=ot_all[0:rows, k, :],
                        )
                g_t[si] = None

            emit_stage_a(0)
            if n_sup > 1:
                emit_stage_a(1)
            for si in range(n_sup):
                if si + 2 < n_sup:
                    emit_stage_a(si + 2)
                emit_stage_b(si)

    nc.compile()
    return nc


# ---------------------------------------------------------------- driver
def prepare(h, W, a, src, dst):
    h = np.asarray(h, dtype=np.float32)
    W = np.asarray(W, dtype=np.float32)
    a = np.asarray(a, dtype=np.float32)
    n_nodes = h.shape[0]
    shard = n_nodes // CORES

    meta, arrs = preprocess_indices(src, dst, n_nodes=n_nodes)
    key = (
        n_nodes,
        h.shape[1],
        W.shape[0],
        tuple(
            (s["cap"], tuple(s["capA"]), tuple(s["capB"])) for s in meta["supers"]
        ),
    )
    if key not in _PROGRAM_CACHE:
        _PROGRAM_CACHE[key] = build_program(
            meta, n_nodes=n_nodes, d_in=h.shape[1], d_out=W.shape[0]
        )
    nc = _PROGRAM_CACHE[key]

    in_maps = []
    for c in range(CORES):
        h_t_c = np.ascontiguousarray(h[c * shard : (c + 1) * shard].T)
        m = {"h_t": h_t_c, "W": W, "a": a}
        m.update(arrs[c])
        in_maps.append(m)
    return nc, in_maps


def kernel(h, W, a, src, dst):
    from concourse.bass_utils import run_bass_kernel_spmd

    nc, in_maps = prepare(h, W, a, src, dst)
    res = run_bass_kernel_spmd(nc, in_maps, core_ids=list(range(CORES)))
    outs = [res.results[c]["out"] for c in range(CORES)]
    return np.ascontiguousarray(np.concatenate(outs, axis=0).astype(np.float32))
